# revision 3
# baseline (speedup 1.0000x reference)
"""Aspheric surface ray intersection on 8 Trainium2 NeuronCores.

Newton iteration (10 steps) per ray to solve z(t) = sag(x(t), y(t)),
embarrassingly data-parallel over 2M rays. Inputs are sharded over the
ray batch dim across 8 cores; scalar surface parameters are baked into
the program as immediates (the Bass program is built per distinct
scalar-parameter set and cached).

Per-core data layout: planar [128, Fc] float32 slabs per component
(ox, oy, oz, dx, dy, dz), Fc rays per partition. Outputs are 7 planar
slabs (t, px, py, pz, nx, ny, nz) re-assembled on the host.
"""

import sys

sys.path.insert(0, "/opt/trn_rl_repo")

import numpy as np

import concourse.bass as bass
import concourse.mybir as mybir
from concourse.tile import TileContext
from concourse.bass_utils import run_bass_kernel_spmd

P = 128
NCORES = 8
MAX_ITER = 10
F32 = np.float32

# tile geometry: per-core rays = P * FC, processed in NTILES tiles of FT
FC = 1956
NTILES = 3
FT = FC // NTILES

AF = mybir.ActivationFunctionType
ALU = mybir.AluOpType

# engine-assignment knobs (tuned against perfetto traces)
GPSIMD_XYZ = True   # move the X/Y/Z Newton-update block to GPSIMD
GPSIMD_RDOT = True  # move w1/w2/rdot to GPSIMD


def _split_sync_waits(nc, max_waits=1):
    """Walrus TPB codegen rejects instructions with more than one sem
    wait. Hoist overflow waits onto NoOps emitted just before, on the
    same engine."""
    n = 0
    for f in nc.m.functions:
        for bb in f.blocks:
            new_insts = []
            for inst in bb.instructions:
                si = getattr(inst, "sync_info", None)
                if si is not None and si.on_wait and len(si.on_wait) > max_waits:
                    waits = list(si.on_wait)
                    head, rest = waits[:-max_waits], waits[-max_waits:]
                    while head:
                        chunk, head = head[:max_waits], head[max_waits:]
                        n += 1
                        new_insts.append(
                            mybir.InstNoOp(
                                name=f"I-waitsplit-{n}",
                                engine=inst.engine,
                                bass_nofuse=True,
                                sync_info=mybir.SyncInfo(on_wait=chunk, on_update=[]),
                            )
                        )
                    inst.sync_info = mybir.SyncInfo(
                        on_wait=rest, on_update=list(si.on_update)
                    )
                new_insts.append(inst)
            bb.instructions = new_insts
    return n


def _sag_scalar(x, y, c, k, a):
    """Host-side fp32 sag at a point (for the z0 offset constant)."""
    x, y, c, k = F32(x), F32(y), F32(c), F32(k)
    r2 = F32(x * x + y * y)
    u = F32(F32(1.0 + k) * c * c)
    s = F32(np.sqrt(F32(1.0 - u * r2)))
    z = F32(r2 * c / F32(1.0 + s))
    q = F32(r2 * r2)
    z = F32(z + q * F32(a[0] + q * F32(a[1] + q * F32(a[2] + q * a[3]))))
    return float(z)


def _build(scal):
    """Build the Bass program for one core-shard. scal is a dict of the
    baked scalar parameters (python floats, already fp32-rounded)."""
    c = scal["c"]
    x0, y0 = scal["x0"], scal["y0"]
    a0, a1, a2, a3 = scal["a"]
    u = scal["u"]
    z0 = scal["z0"]
    half_c = float(F32(0.5 * F32(c)))

    nc = bass.Bass("TRN2", target_bir_lowering=False, debug=False)
    dt = mybir.dt.float32

    ins = {
        name: nc.declare_dram_parameter(name, [P, FC], dt, isOutput=False)
        for name in ["ox", "oy", "oz", "dx", "dy", "dz"]
    }
    outs = {
        name: nc.declare_dram_parameter(name, [P, FC], dt, isOutput=True)
        for name in ["t", "px", "py", "pz", "nx", "ny", "nz"]
    }

    with TileContext(nc) as tc:
        with (
            tc.tile_pool(name="state", bufs=NTILES) as state,
            tc.tile_pool(name="tmp", bufs=2 * NTILES) as tmp,
            tc.tile_pool(name="io", bufs=2 * NTILES) as io,
            tc.tile_pool(name="const", bufs=1) as constp,
        ):
            inf_tile = constp.tile([P, FT], dt, tag="inf", name="inf")
            nc.vector.memset(inf_tile[:], float("inf"))

            for ti in range(NTILES):
                sl = bass.ts(ti, FT)

                def dma_in(name, tag, pool=io):
                    t = pool.tile([P, FT], dt, tag=tag)
                    nc.sync.dma_start(out=t[:], in_=ins[name][:, sl])
                    return t

                t_ox = dma_in("ox", "ld_ox")
                t_oy = dma_in("oy", "ld_oy")
                t_oz = dma_in("oz", "ld_oz")
                t_dx = state.tile([P, FT], dt, tag="dx", name="dx")
                nc.sync.dma_start(out=t_dx[:], in_=ins["dx"][:, sl])
                t_dy = state.tile([P, FT], dt, tag="dy", name="dy")
                nc.sync.dma_start(out=t_dy[:], in_=ins["dy"][:, sl])
                t_dz = state.tile([P, FT], dt, tag="dz", name="dz")
                nc.sync.dma_start(out=t_dz[:], in_=ins["dz"][:, sl])

                def T(tag, pool=tmp):
                    return pool.tile([P, FT], dt, tag=tag, name=tag)

                # ---- prologue: t0 = relu(-oz/dz); X/Y/Z at t0 ----
                rdz = state.tile([P, FT], dt, tag="rdz", name="rdz")
                nc.vector.reciprocal(rdz[:], t_dz[:])
                ozr = T("ozr")
                nc.vector.tensor_tensor(ozr[:], t_oz[:], rdz[:], ALU.mult)
                t0 = T("t0")
                nc.scalar.activation(t0[:], ozr[:], AF.Relu, scale=-1.0)

                oxp = T("oxp")
                nc.scalar.activation(oxp[:], t_ox[:], AF.Copy, bias=x0)
                oyp = T("oyp")
                nc.scalar.activation(oyp[:], t_oy[:], AF.Copy, bias=y0)
                ozp = state.tile([P, FT], dt, tag="ozp", name="ozp")
                nc.scalar.activation(ozp[:], t_oz[:], AF.Copy, bias=z0)

                X = state.tile([P, FT], dt, tag="X", name="X")
                Y = state.tile([P, FT], dt, tag="Y", name="Y")
                Z = state.tile([P, FT], dt, tag="Z", name="Z")
                # X = oxp + t0*dx  (STT: (t0 * 1) * dx, then add)
                w = T("w0")
                nc.vector.tensor_tensor(w[:], t0[:], t_dx[:], ALU.mult)
                nc.vector.tensor_tensor(X[:], oxp[:], w[:], ALU.add)
                w = T("w0")
                nc.vector.tensor_tensor(w[:], t0[:], t_dy[:], ALU.mult)
                nc.vector.tensor_tensor(Y[:], oyp[:], w[:], ALU.add)
                w = T("w0")
                nc.vector.tensor_tensor(w[:], t0[:], t_dz[:], ALU.mult)
                nc.vector.tensor_tensor(Z[:], ozp[:], w[:], ALU.add)

                def sag_eval(need_deriv):
                    """Emit sag-evaluation ops at current X,Y,Z.
                    Returns dict of result tiles."""
                    x2 = T("x2")
                    nc.scalar.activation(x2[:], X[:], AF.Square)
                    y2 = T("y2")
                    nc.scalar.activation(y2[:], Y[:], AF.Square)
                    r2 = T("r2")
                    nc.vector.tensor_tensor(r2[:], x2[:], y2[:], ALU.add)
                    q = T("q")
                    nc.scalar.activation(q[:], r2[:], AF.Square)
                    q2 = T("q2")
                    nc.scalar.activation(q2[:], q[:], AF.Square)
                    # p = q*(A1 + A2*q2), A1 = a0 + a1*q, A2 = a2 + a3*q
                    A1 = T("A1")
                    nc.scalar.activation(A1[:], q[:], AF.Copy, bias=a0, scale=a1)
                    A2 = T("A2")
                    nc.scalar.activation(A2[:], q[:], AF.Copy, bias=a2, scale=a3)
                    B = T("B")
                    nc.vector.tensor_tensor(B[:], A2[:], q2[:], ALU.mult)
                    C = T("C")
                    nc.vector.tensor_tensor(C[:], A1[:], B[:], ALU.add)
                    p = T("p")
                    nc.vector.tensor_tensor(p[:], C[:], q[:], ALU.mult)
                    res = {"r2": r2, "q": q, "q2": q2, "p": p}
                    # zp = sag_raw = (c/2 or c*r2/(1+s)) + p
                    zp = T("zp")
                    if u == 0.0:
                        nc.vector.scalar_tensor_tensor(
                            zp[:], r2[:], half_c, p[:], ALU.mult, ALU.add
                        )
                    else:
                        s = T("s")
                        nc.scalar.activation(s[:], r2[:], AF.Sqrt, bias=1.0, scale=-u)
                        sp1 = T("sp1")
                        nc.scalar.activation(sp1[:], s[:], AF.Copy, bias=1.0)
                        rec = T("rec")
                        nc.vector.reciprocal(rec[:], sp1[:])
                        zc = T("zc")
                        nc.vector.scalar_tensor_tensor(
                            zc[:], r2[:], float(F32(c)), rec[:], ALU.mult, ALU.mult
                        )
                        nc.vector.tensor_tensor(zp[:], zc[:], p[:], ALU.add)
                        res["s"] = s
                    res["zp"] = zp
                    if need_deriv:
                        # f2 = 2*f' ; u=0: f2 = c + 4*r2*D,
                        # else   f2 = c/s + 4*r2*D  with D = D1 + D2*q2
                        D1 = T("D1")
                        nc.scalar.activation(
                            D1[:], q[:], AF.Copy, bias=a0, scale=2.0 * a1
                        )
                        D2 = T("D2")
                        nc.scalar.activation(
                            D2[:], q[:], AF.Copy, bias=3.0 * a2, scale=4.0 * a3
                        )
                        E = T("E")
                        nc.vector.tensor_tensor(E[:], D2[:], q2[:], ALU.mult)
                        Dv = T("Dv")
                        nc.vector.tensor_tensor(Dv[:], D1[:], E[:], ALU.add)
                        e = T("e")
                        nc.vector.tensor_tensor(e[:], Dv[:], r2[:], ALU.mult)
                        f2 = T("f2")
                        if u == 0.0:
                            nc.scalar.activation(
                                f2[:], e[:], AF.Copy, bias=c, scale=4.0
                            )
                        else:
                            e4 = T("e4")
                            nc.scalar.activation(e4[:], e[:], AF.Copy, scale=4.0)
                            rs = T("rs")
                            nc.vector.reciprocal(rs[:], res["s"][:])
                            nc.vector.scalar_tensor_tensor(
                                f2[:], rs[:], float(F32(c)), e4[:], ALU.mult, ALU.add
                            )
                        res["f2"] = f2
                    return res

                # ---- Newton iterations ----
                for it in range(MAX_ITER):
                    sv = sag_eval(need_deriv=True)
                    g = T("g")
                    nc.vector.tensor_tensor(g[:], Z[:], sv["zp"][:], ALU.subtract)
                    eng_r = nc.gpsimd if GPSIMD_RDOT else nc.vector
                    w1 = T("w1")
                    eng_r.tensor_tensor(w1[:], X[:], t_dx[:], ALU.mult)
                    w2 = T("w2")
                    eng_r.tensor_tensor(w2[:], Y[:], t_dy[:], ALU.mult)
                    rdot = T("rdot")
                    eng_r.tensor_tensor(rdot[:], w1[:], w2[:], ALU.add)
                    m = T("m")
                    nc.vector.tensor_tensor(m[:], rdot[:], sv["f2"][:], ALU.mult)
                    gp = T("gp")
                    nc.vector.tensor_tensor(gp[:], t_dz[:], m[:], ALU.subtract)
                    rgp = T("rgp")
                    nc.vector.reciprocal(rgp[:], gp[:])
                    delta = T("delta")
                    nc.vector.tensor_tensor(delta[:], g[:], rgp[:], ALU.mult)
                    eng_u = nc.gpsimd if GPSIMD_XYZ else nc.vector
                    for coord, d in ((X, t_dx), (Y, t_dy), (Z, t_dz)):
                        mv = T("mv")
                        eng_u.tensor_tensor(mv[:], delta[:], d[:], ALU.mult)
                        eng_u.tensor_tensor(coord[:], coord[:], mv[:], ALU.subtract)

                # ---- epilogue ----
                sv = sag_eval(need_deriv=True)
                g = T("g")
                nc.vector.tensor_tensor(g[:], Z[:], sv["zp"][:], ALU.subtract)
                res_t = T("resid")
                nc.scalar.activation(res_t[:], g[:], AF.Abs)
                # t = (Z - ozp) * rdz
                tt1 = T("tt1")
                nc.vector.tensor_tensor(tt1[:], Z[:], ozp[:], ALU.subtract)
                t_fin = T("tfin")
                nc.vector.tensor_tensor(t_fin[:], tt1[:], rdz[:], ALU.mult)
                # valid = (t > 1e-8) & (res < 1e-3); t_out = valid ? t : inf
                c1 = T("c1")
                nc.vector.tensor_scalar(c1[:], t_fin[:], 1e-8, None, ALU.is_gt)
                vmask = T("vmask")
                nc.vector.scalar_tensor_tensor(
                    vmask[:], res_t[:], 1e-3, c1[:], ALU.is_lt, ALU.mult
                )
                t_out = io.tile([P, FT], dt, tag="st_t", name="st_t")
                nc.vector.select(
                    t_out[:], vmask[:].bitcast(mybir.dt.uint32), t_fin[:], inf_tile[:]
                )
                nc.sync.dma_start(out=outs["t"][:, sl], in_=t_out[:])
                # point
                xh = io.tile([P, FT], dt, tag="st_px", name="st_px")
                nc.scalar.activation(xh[:], X[:], AF.Copy, bias=-x0)
                nc.sync.dma_start(out=outs["px"][:, sl], in_=xh[:])
                yh = io.tile([P, FT], dt, tag="st_py", name="st_py")
                nc.scalar.activation(yh[:], Y[:], AF.Copy, bias=-y0)
                nc.sync.dma_start(out=outs["py"][:, sl], in_=yh[:])
                zh = io.tile([P, FT], dt, tag="st_pz", name="st_pz")
                nc.scalar.activation(zh[:], sv["zp"][:], AF.Copy, bias=-z0)
                nc.sync.dma_start(out=outs["pz"][:, sl], in_=zh[:])
                # normal: n = (-f'*2X, -f'*2Y, 1)/norm ; f2 = 2f'
                f2n = T("f2n")
                nc.scalar.activation(f2n[:], sv["f2"][:], AF.Copy, scale=-1.0)
                dzdxn = T("dzdxn")
                nc.vector.tensor_tensor(dzdxn[:], f2n[:], X[:], ALU.mult)
                dzdyn = T("dzdyn")
                nc.vector.tensor_tensor(dzdyn[:], f2n[:], Y[:], ALU.mult)
                w1s = T("w1s")
                nc.scalar.activation(w1s[:], dzdxn[:], AF.Square)
                w2s = T("w2s")
                nc.scalar.activation(w2s[:], dzdyn[:], AF.Square)
                w3 = T("w3")
                nc.vector.tensor_tensor(w3[:], w1s[:], w2s[:], ALU.add)
                sn = T("sn")
                nc.scalar.activation(sn[:], w3[:], AF.Sqrt, bias=1.0)
                rn = io.tile([P, FT], dt, tag="st_nz", name="st_nz")
                nc.vector.reciprocal(rn[:], sn[:])
                nx = io.tile([P, FT], dt, tag="st_nx", name="st_nx")
                nc.vector.tensor_tensor(nx[:], dzdxn[:], rn[:], ALU.mult)
                nc.sync.dma_start(out=outs["nx"][:, sl], in_=nx[:])
                ny = io.tile([P, FT], dt, tag="st_ny", name="st_ny")
                nc.vector.tensor_tensor(ny[:], dzdyn[:], rn[:], ALU.mult)
                nc.sync.dma_start(out=outs["ny"][:, sl], in_=ny[:])
                nc.sync.dma_start(out=outs["nz"][:, sl], in_=rn[:])

    _split_sync_waits(nc)
    return nc


_nc_cache = {}


def _get_program(scal):
    key = tuple(sorted((k, tuple(v) if isinstance(v, tuple) else v)
                       for k, v in scal.items()))
    if key not in _nc_cache:
        _nc_cache[key] = _build(scal)
    return _nc_cache[key]


def _run(ray_origin, ray_direction, scal, trace=False):
    N = ray_origin.shape[0]
    R = P * FC
    Npad = NCORES * R
    ro = np.ascontiguousarray(np.asarray(ray_origin, dtype=np.float32))
    rd = np.ascontiguousarray(np.asarray(ray_direction, dtype=np.float32))
    if Npad > N:
        pad_o = np.tile(np.array([0.0, 0.0, -100.0], np.float32), (Npad - N, 1))
        pad_d = np.tile(np.array([0.0, 0.0, 1.0], np.float32), (Npad - N, 1))
        ro = np.concatenate([ro, pad_o], axis=0)
        rd = np.concatenate([rd, pad_d], axis=0)

    in_maps = []
    for ci in range(NCORES):
        so = ro[ci * R : (ci + 1) * R]
        sd = rd[ci * R : (ci + 1) * R]
        in_maps.append(
            {
                "ox": np.ascontiguousarray(so[:, 0].reshape(P, FC)),
                "oy": np.ascontiguousarray(so[:, 1].reshape(P, FC)),
                "oz": np.ascontiguousarray(so[:, 2].reshape(P, FC)),
                "dx": np.ascontiguousarray(sd[:, 0].reshape(P, FC)),
                "dy": np.ascontiguousarray(sd[:, 1].reshape(P, FC)),
                "dz": np.ascontiguousarray(sd[:, 2].reshape(P, FC)),
            }
        )

    nc = _get_program(scal)
    res = run_bass_kernel_spmd(
        nc, in_maps, core_ids=list(range(NCORES)), trace=trace
    )

    def gather(name):
        return np.concatenate(
            [res.results[ci][name].reshape(R) for ci in range(NCORES)]
        )[:N]

    t_out = gather("t")
    point = np.stack([gather("px"), gather("py"), gather("pz")], axis=-1)
    normal = np.stack([gather("nx"), gather("ny"), gather("nz")], axis=-1)
    return (t_out, point, normal), res


def _scalars(offset, curvature, conic, aspheric):
    off = np.asarray(offset, dtype=np.float32)
    c = float(F32(np.asarray(curvature).item()))
    k = float(F32(np.asarray(conic).item()))
    a = tuple(float(F32(v)) for v in np.asarray(aspheric, dtype=np.float32))
    u = float(F32(F32(1.0 + F32(k)) * F32(c) * F32(c)))
    z0 = _sag_scalar(off[0], off[1], c, k, a)
    return {
        "c": c,
        "x0": float(off[0]),
        "y0": float(off[1]),
        "a": a,
        "u": u,
        "z0": z0,
    }


def kernel(ray_origin, ray_direction, offset, curvature, conic, aspheric):
    scal = _scalars(offset, curvature, conic, aspheric)
    out, _ = _run(ray_origin, ray_direction, scal)
    return out


def kernel_with_stats(ray_origin, ray_direction, offset, curvature, conic, aspheric):
    """Like kernel() but also profiles the NEFF; returns (out, exec_time_ns)."""
    try:
        sys.path.insert(0, "/tmp")
        import ntff_hook

        ntff_hook.install()
    except Exception as e:
        print("ntff hook unavailable:", e)
    scal = _scalars(offset, curvature, conic, aspheric)
    out, res = _run(ray_origin, ray_direction, scal, trace=True)
    return out, res.exec_time_ns
